# revision 8
# baseline (speedup 1.0000x reference)
"""Paged multi-head attention decode step on 8 trn2 NeuronCores.

Sharding: tensor-parallel over heads. Core c owns heads [4c, 4c+4):
  - rows  [512c, 512(c+1)) of Wq/Wk/Wv, cols [512c, 512(c+1)) of Wo
  - head-slice of the (gathered, per-sequence) KV cache
Each core computes q/k/v for its heads for all 8 sequences, injects the new
token's k/v into its KV tiles, runs softmax(q K^T / sqrt(d)) V over the valid
context, then a partial output projection out_c = ctx_c @ Wo_c.  The full
output is the sum over cores (done on host).

All HBM-streamed operands (x, Wq/Wk/Wv/Wo slices, gathered K/V) are cast to
bf16 on the host — halves HBM traffic (the binding resource; ~30 MB/core) and
enables PE fast-weight-load.  PSUM accumulation stays fp32; softmax
denominators and the context normalization stay fp32.

PE cost shape (ldweights ~ columns/1.2 GHz, so the stationary operand must be
the FEW-column one wherever possible):
  - projections: x^T chunk [128k, 8b] stationary (8-col LDW), W^T chunk
    [128k, 512j] moving -> q/k/v land row-major [8, 512] in psum; cheap PE
    transposes produce the column forms attention needs.
  - scores: K chunk [128d, <=128t] stationary (bf16 FWL), q column moving;
    out [tok, 4 h] -> exp -> bf16 attn tile.  The final tile's contraction
    is the EXACT number of valid tokens, so no masking is ever needed.
  - PV: attn [tok, 4h] stationary (4-col LDW), V [tok, 512] moving; a second
    N=1 matmul against a constant ones vector accumulates the softmax
    denominators (no reduce / transpose / broadcast chain).
  - Wo: ctxT [128d, 8b] stationary (8-col LDW), Wo^T [128d, 512j] moving.

Sequences are processed longest-first so the post-last-DMA tail is short.
Sequence lengths (positions) are host-known at trace time, so all loop trip
counts are static and the kernel only reads the valid context.
"""

import math

import numpy as np
import ml_dtypes

import concourse.bass as bass
import concourse.mybir as mybir
import concourse.tile as tile
from concourse import bacc
from concourse.bass_utils import run_bass_kernel_spmd
from concourse.masks import make_identity

BLOCK_SIZE = 16
NUM_HEADS = 32
HEAD_DIM = 128
D_MODEL = NUM_HEADS * HEAD_DIM
B = 8
N_CORES = 8
H_LOC = NUM_HEADS // N_CORES          # 4 heads per core
KSLICE = H_LOC * HEAD_DIM             # 512 contraction slice per core
NPAIR = H_LOC * B                     # 32 (seq, head) pairs per core
SCALE = 1.0 / math.sqrt(HEAD_DIM)

_F32 = mybir.dt.float32
_BF16 = mybir.dt.bfloat16


def _cfg_from_positions(pos):
    pos = [int(p) for p in pos]
    tv = [p + 1 for p in pos]                      # valid context lengths
    tva = [(t + 7) // 8 * 8 for t in tv]           # K col alignment per head
    nt = [(t + 127) // 128 for t in tv]
    kofs = np.concatenate([[0], np.cumsum([4 * t for t in tva])]).tolist()
    vofs = np.concatenate([[0], np.cumsum([512 * n for n in nt])]).tolist()
    return {
        "pos": pos, "tv": tv, "tva": tva, "nt": nt,
        "kofs": kofs, "vofs": vofs,
        "sumk": int(kofs[-1]), "sumv": int(vofs[-1]),
        "maxnt": max(nt), "maxtva": max(tva),
        "order": sorted(range(B), key=lambda b: -tv[b]),
    }


def _build(cfg, nrep=1):
    pos, tv, tva, nt = cfg["pos"], cfg["tv"], cfg["tva"], cfg["nt"]
    kofs, vofs = cfg["kofs"], cfg["vofs"]
    maxnt, maxtva = cfg["maxnt"], cfg["maxtva"]

    nc = bacc.Bacc("TRN2", target_bir_lowering=False, debug=False)

    xt_d = nc.dram_tensor("xt", [128, 32 * B], _BF16, kind="ExternalInput")
    wqkv_d = nc.dram_tensor("wqkv", [128, 3 * 32 * 512], _BF16, kind="ExternalInput")
    wo_d = nc.dram_tensor("wo_t", [128, 32 * 512], _BF16, kind="ExternalInput")
    kt_d = nc.dram_tensor("kt", [128, cfg["sumk"]], _BF16, kind="ExternalInput")
    vg_d = nc.dram_tensor("vg", [128, cfg["sumv"]], _BF16, kind="ExternalInput")
    out_d = nc.dram_tensor("out_part", [B, D_MODEL], _F32, kind="ExternalOutput")

    with tile.TileContext(nc) as tc:
        with (
            tc.tile_pool(name="const", bufs=1) as const,
            tc.tile_pool(name="wstream", bufs=3) as wpool,
            tc.tile_pool(name="wostream", bufs=4) as wopool,
            tc.tile_pool(name="kstream", bufs=3) as kpool,
            tc.tile_pool(name="vstream", bufs=3) as vpool,
            tc.tile_pool(name="ps", bufs=8, space="PSUM") as psp,
        ):
            ident = const.tile([8, 8], _F32, tag="ident")
            make_identity(nc, ident[:])
            ident_bf = const.tile([4, 4], _BF16, tag="ident_bf")
            make_identity(nc, ident_bf[:])
            ones_bf = const.tile([128, 1], _BF16, tag="ones_bf")
            nc.vector.memset(ones_bf[:], 1.0)

            for rep in range(nrep):
                xt_sb = const.tile([128, 32, B], _BF16, tag="xt")
                nc.sync.dma_start(
                    out=xt_sb[:], in_=xt_d.ap().rearrange("p (c b) -> p c b", b=B)
                )

                # ---- Q,K,V projections, classic form: x^T chunk stationary
                # (8-col LDW), W^T chunk moving (512 cols).  q/k/v land
                # row-major [8 b, 512 (h d)] in psum, accumulated over the 32
                # k-chunks streaming in 4 x 1MB DMAs per projection.
                rows = []  # q_sb, k_sb (f32 row form), v_bf (bf16 row form)
                for p_i, pname in enumerate(("q", "k", "v")):
                    ps = psp.tile([B, KSLICE], _F32, tag="ps", name=f"ps_{pname}")
                    for g in range(4):
                        wt = wpool.tile([128, 4096], _BF16, tag="w",
                                        name=f"wt_{pname}{g}")
                        nc.sync.dma_start(
                            out=wt[:],
                            in_=wqkv_d.ap()[:, (4 * p_i + g) * 4096:
                                            (4 * p_i + g + 1) * 4096],
                        )
                        for j in range(8):
                            i = 8 * g + j
                            nc.tensor.matmul(
                                ps[:], lhsT=xt_sb[:, i, :],
                                rhs=wt[:, 512 * j: 512 * (j + 1)],
                                start=(i == 0), stop=(i == 31),
                            )
                    if pname == "v":
                        v_bf = const.tile([B, KSLICE], _BF16, tag="v_bf")
                        nc.scalar.copy(out=v_bf[:], in_=ps[:])
                        rows.append(v_bf)
                    else:
                        row = const.tile([B, KSLICE], _F32, tag=f"{pname}_sb")
                        nc.vector.tensor_copy(out=row[:], in_=ps[:])
                        rows.append(row)
                q_sb, k_sb, v_bf = rows

                # ---- transpose q,k row-form -> column form [128 d, 8h+b] bf16
                qT = const.tile([128, NPAIR], _BF16, tag="qT")
                kT = const.tile([128, NPAIR], _BF16, tag="kT")
                for src, dst in ((q_sb, qT), (k_sb, kT)):
                    for h in range(H_LOC):
                        tp = psp.tile([128, B], _F32, tag="ps", name=f"tp{h}")
                        nc.tensor.transpose(
                            tp[:], src[0:B, 128 * h: 128 * (h + 1)], ident[:]
                        )
                        nc.vector.tensor_copy(
                            out=dst[:, 8 * h: 8 * h + B], in_=tp[:]
                        )

                # ---- attention, streamed per sequence, longest first
                # (one-pass softmax; scores are O(1) so exp needs no shift)
                ctx_bf = const.tile([128, NPAIR], _BF16, tag="ctx_bf")  # 8h+b
                for b in cfg["order"]:
                    kt_t = kpool.tile([128, H_LOC, maxtva], _BF16, tag="kt",
                                      name=f"kt{b}")
                    nc.sync.dma_start(
                        out=kt_t[:, :, 0:tva[b]],
                        in_=kt_d.ap()[:, kofs[b]: kofs[b] + 4 * tva[b]]
                        .rearrange("p (h t) -> p h t", h=H_LOC),
                    )
                    vt = vpool.tile([128, maxnt, 512], _BF16, tag="vt",
                                    name=f"vt{b}")
                    nc.sync.dma_start(
                        out=vt[:, 0:nt[b], :],
                        in_=vg_d.ap()[:, vofs[b]: vofs[b] + 512 * nt[b]]
                        .rearrange("p (c f) -> p c f", f=512),
                    )
                    # inject the new token's k (column pos) and v (row pos)
                    nc.vector.tensor_copy(
                        out=kt_t[:, :, pos[b]],
                        in_=kT[:].rearrange("p (h b) -> p b h", b=B)[:, b, :],
                    )
                    nc.sync.dma_start(
                        out=vt[pos[b] % 128: pos[b] % 128 + 1, nt[b] - 1, :],
                        in_=v_bf[b: b + 1, :],
                    )

                    attn_b = kpool.tile([128, nt[b], H_LOC], _BF16, tag="attn",
                                        name=f"attn{b}", bufs=2)
                    ct = psp.tile([H_LOC, KSLICE], _F32, tag="ps", name=f"ct{b}")
                    dn = psp.tile([H_LOC, 1], _F32, tag="ps", name=f"dn{b}")
                    for tt in range(nt[b]):
                        w = min(128, tv[b] - 128 * tt)
                        sc = psp.tile([128, H_LOC], _F32, tag="ps",
                                      name=f"sc{b}_{tt}")
                        for h in range(H_LOC):
                            nc.tensor.matmul(
                                sc[0:w, h: h + 1],
                                lhsT=kt_t[:, h, 128 * tt: 128 * tt + w],
                                rhs=qT[:, 8 * h + b: 8 * h + b + 1],
                                start=(h == 0), stop=(h == H_LOC - 1),
                            )
                        nc.scalar.activation(
                            out=attn_b[0:w, tt, :], in_=sc[0:w, :],
                            func=mybir.ActivationFunctionType.Exp,
                        )
                        nc.tensor.matmul(
                            ct[:],
                            lhsT=attn_b[0:w, tt, :],
                            rhs=vt[0:w, tt, :],
                            start=(tt == 0), stop=(tt == nt[b] - 1),
                        )
                        nc.tensor.matmul(
                            dn[:],
                            lhsT=attn_b[0:w, tt, :],
                            rhs=ones_bf[0:w, :],
                            start=(tt == 0), stop=(tt == nt[b] - 1),
                        )
                    rc = const.tile([H_LOC, 1], _F32, tag="rc", name=f"rc{b}",
                                    bufs=2)
                    nc.vector.reciprocal(rc[:], dn[:])
                    ct_bf = const.tile([H_LOC, KSLICE], _BF16, tag="ct_bf",
                                       name=f"ct_bf{b}", bufs=2)
                    nc.vector.tensor_scalar_mul(ct_bf[:], ct[:], rc[:])
                    for h in range(H_LOC):
                        ctt = psp.tile([128, H_LOC], _BF16, tag="ps",
                                       name=f"ctt{b}_{h}")
                        nc.tensor.transpose(
                            ctt[:], ct_bf[0:H_LOC, 128 * h: 128 * (h + 1)],
                            ident_bf[:],
                        )
                        nc.vector.tensor_copy(
                            out=ctx_bf[:, 8 * h + b: 8 * h + b + 1],
                            in_=ctt[:, h: h + 1],
                        )

                # ---- output projection partial:
                # out[b, 512n+j] = sum_h ctx_bf[:, 8h+b]^T wo[:, (n,h,j)]
                outsb = const.tile([B, D_MODEL], _F32, tag="outsb")
                for g in range(4):
                    wot = wopool.tile([128, 4096], _BF16, tag="wo", name=f"wo{g}")
                    nc.sync.dma_start(
                        out=wot[:], in_=wo_d.ap()[:, g * 4096: (g + 1) * 4096]
                    )
                    for m in range(2):
                        n = 2 * g + m
                        op = psp.tile([B, 512], _F32, tag="ps", name=f"op{n}")
                        for h in range(H_LOC):
                            nc.tensor.matmul(
                                op[:],
                                lhsT=ctx_bf[:, 8 * h: 8 * h + B],
                                rhs=wot[:, 2048 * m + 512 * h:
                                        2048 * m + 512 * (h + 1)],
                                start=(h == 0), stop=(h == H_LOC - 1),
                            )
                        nc.scalar.copy(
                            out=outsb[:, 512 * n: 512 * (n + 1)], in_=op[:]
                        )
                nc.sync.dma_start(out=out_d.ap(), in_=outsb[:])

    nc.compile()
    return nc


_PROGRAM_CACHE = {}


def _get_program(cfg, nrep=1):
    key = (tuple(cfg["pos"]), nrep)
    if key not in _PROGRAM_CACHE:
        _PROGRAM_CACHE[key] = _build(cfg, nrep=nrep)
    return _PROGRAM_CACHE[key]


def _bf(a):
    return np.asarray(a, dtype=ml_dtypes.bfloat16)


def make_core_inputs(cfg, c, x, Wq, Wk, Wv, Wo, key_cache, value_cache,
                     block_tables):
    """Host-side shard prep for core c."""
    tv, tva, nt = cfg["tv"], cfg["tva"], cfg["nt"]
    h0 = H_LOC * c
    ksl = slice(KSLICE * c, KSLICE * (c + 1))

    xt = _bf(x.reshape(B, D_MODEL).T.reshape(32, 128, B)
             .transpose(1, 0, 2).reshape(128, 32 * B))

    def wrow(W, scale=1.0):
        # W_slice^T [4096 k, 512 j] -> [128, 32*512] with 32 k-chunks packed
        # contiguously along each partition row
        return (W[ksl, :].T * scale).reshape(32, 128, 512).transpose(1, 0, 2) \
            .reshape(128, 32 * 512)

    wqkv = _bf(np.concatenate(
        [wrow(Wq, SCALE), wrow(Wk), wrow(Wv)], axis=1))

    # Wo^T slice [512 k, 4096 j] -> [128 d, (8 n, 4 h, 512 j)]
    wo_t = _bf(Wo[:, ksl].T.reshape(H_LOC, 128, 8, 512)
               .transpose(1, 2, 0, 3).reshape(128, 32 * 512))

    kt = np.zeros((128, cfg["sumk"]), dtype=ml_dtypes.bfloat16)
    vg = np.zeros((128, cfg["sumv"]), dtype=ml_dtypes.bfloat16)
    for b in range(B):
        ntok = 128 * nt[b]
        nb = (ntok + BLOCK_SIZE - 1) // BLOCK_SIZE
        blocks = np.asarray(block_tables[b, :nb])
        kb = key_cache[blocks][:, :, h0: h0 + H_LOC, :].reshape(
            ntok, H_LOC, HEAD_DIM)[:tv[b]]
        vb = value_cache[blocks][:, :, h0: h0 + H_LOC, :].reshape(
            ntok, H_LOC, HEAD_DIM)
        # K: [tv, 4h, 128d] -> [128 d, (4 h, tva t)], pad cols zero
        ktb = np.zeros((HEAD_DIM, H_LOC, tva[b]), np.float32)
        ktb[:, :, :tv[b]] = kb.transpose(2, 1, 0)
        kt[:, cfg["kofs"][b]: cfg["kofs"][b] + 4 * tva[b]] = _bf(
            ktb.reshape(HEAD_DIM, H_LOC * tva[b]))
        # V: [128nt, 512] -> [128 p, (nt c, 512 f)]
        vg[:, cfg["vofs"][b]: cfg["vofs"][b] + 512 * nt[b]] = _bf(
            vb.reshape(nt[b], 128, KSLICE).transpose(1, 0, 2)
            .reshape(128, nt[b] * KSLICE))
    return {"xt": xt, "wqkv": wqkv, "wo_t": wo_t, "kt": kt, "vg": vg}


def kernel(x, Wq, Wk, Wv, Wo, key_cache, value_cache, block_tables, positions,
           _trace=False):
    x = np.asarray(x, dtype=np.float32)
    Wq = np.asarray(Wq, dtype=np.float32)
    Wk = np.asarray(Wk, dtype=np.float32)
    Wv = np.asarray(Wv, dtype=np.float32)
    Wo = np.asarray(Wo, dtype=np.float32)
    key_cache = np.asarray(key_cache, dtype=np.float32)
    value_cache = np.asarray(value_cache, dtype=np.float32)
    block_tables = np.asarray(block_tables)
    positions = np.asarray(positions)

    cfg = _cfg_from_positions(positions)
    nc = _get_program(cfg)

    in_maps = [
        make_core_inputs(cfg, c, x, Wq, Wk, Wv, Wo, key_cache, value_cache,
                         block_tables)
        for c in range(N_CORES)
    ]
    res = run_bass_kernel_spmd(nc, in_maps, core_ids=list(range(N_CORES)))
    out = np.zeros((B, D_MODEL), dtype=np.float32)
    for r in res.results:
        out += r["out_part"]
    kernel.last_results = res
    return out.reshape(B, 1, D_MODEL).astype(np.float32)
